# revision 32
# baseline (speedup 1.0000x reference)
"""Batch semi-hard triplet loss on 8 Trainium2 NeuronCores (Bass/Tile).

Strategy (anchor-row sharding, per sharding hint):
  - Host sorts rows by label (loss is permutation invariant), computes row
    norms and per-row class-block bounds [c0, c1) in sorted order.
  - Each core mines its [1024, 8192] stripe of u = 2*x_i.x_j - |x_j|^2
    (sq dist = |x_i|^2 - u).  Work is split across THREE engines per
    128-row block, 2048-col macro chunk:
      * window phase (hardest positive): PE window matmul + rank-1 norm,
        one custom-DVE TRIPLET_WINMAX per block over the class window
        [c0, c1) (self column included: it only lowers hp_sq to exactly 0
        for singleton classes, which the host invalidates anyway).
        hp_t[b] = max over window of -u, so uL = -hp_t = min u over class.
      * x-chunks (one fixed 2048-col chunk per core, chosen away from the
        block-diagonal): custom-DVE TRIPLET_MAXLT2 reads the dot-only PSUM
        and adds the column norms via its elementwise Src1 stream
        (select(Src0+Src1 < -C0) -> max).  No rank-1 matmul needed.
      * z-chunks (the rest): PE rank-1 completes u in PSUM, the Act engine
        computes w = Ln(uL - u) (excluded cols u >= uL give NaN, which the
        DVE reduce suppresses - hardware-verified semantics), and the DVE
        folds w with a 2x-mode tensor_scalar(min,min) accum chained across
        chunks.  Host decodes gap = exp(min w), maxLT = uL - gap.
  - Device returns per-row (hp_t, x-chunk maxLT, z-chunk min-ln-gap).
    Host finishes the tiny per-row math; rows whose decoded gap is
    suspiciously tiny (exact-hit/leak degenerates) and rows with an empty
    semi-hard band are recomputed exactly on the host (~tens of rows).
"""

import os
import re
import sys

for _p in (
    "/root/.axon_site/_ro/trn_rl_repo/concourse",
    "/root/.axon_site/_ro/trn_rl_repo",
    "/root/.axon_site/_ro/pypackages",
):
    if _p not in sys.path:
        sys.path.insert(0, _p)

from contextlib import ExitStack

import numpy as np

import mybir
import concourse.bass as bass
import concourse.bacc as bacc
import concourse.tile as tile
from concourse.bass_utils import run_bass_kernel_spmd
from concourse import dve_ops as _dops
from concourse.dve_spec import (
    C0, C1, C2, C3, Idx, MaxNeg, Spec, Src0, Src1, Zero, maxx, minn, select,
    _spill_c3_to_src1,
)
from concourse.dve_table_gen import dve_ver_for

B = 8192
D = 128
NCORES = 8
ROWS = B // NCORES        # rows per core
PB = 128                  # rows per block (partition dim)
NB = ROWS // PB           # blocks per core
CH = 512                  # one PSUM bank of f32 (matmul moving max)
MCH = 2048                # macro chunk (4 banks)
NM = B // MCH             # macro chunks across the full column range
W = 256                   # positive-mining window (auto-doubles if needed)
MARGIN = 0.3
NEG_INIT = -3.0e38
FMAX = float(np.finfo(np.float32).max)
LN_NONE = 20.0            # min-ln above this => no semi-hard candidate
LN_TINY = -25.0           # min-ln below this => host rescue (coincidence)
EPS = 1e-3                # guard band: Act arg = (uL - EPS) - u, so the
                          # bit-exact min-positive (u == uL) lands at -EPS
                          # -> NaN -> suppressed; host adds EPS back.

F32 = mybir.dt.float32
AX = mybir.AxisListType
ALU = mybir.AluOpType
ACT = mybir.ActivationFunctionType

_PROGRAM_CACHE = {}

# ---------------------------------------------------------------------------
# custom DVE ops
# ---------------------------------------------------------------------------


def _rowmax(body, init):
    m = body.reshape(body.shape[0], -1).max(axis=-1, keepdims=True)
    return np.maximum(np.asarray(init, np.float32).reshape(-1, 1) * np.ones_like(m), m)


def _ref_maxlt2(in0, in1, c0, c1, imm2):
    u = (in0.astype(np.float32) + in1.astype(np.float32)).astype(np.float32)
    thr = (-np.asarray(c0, np.float32)).reshape(-1, 1)
    body = np.where(u < thr, u, -FMAX).astype(np.float32)
    return body, _rowmax(body, c1)


def _ref_winmax(in0, in1, c0, c1, imm2):
    # in1 carries the spilled C3 (mask end), [P, 1]
    u = in0.astype(np.float32)
    c3 = in1.reshape(in1.shape[0], 1)
    idx = np.arange(u.shape[-1], dtype=np.float32)
    mask = (idx >= np.minimum(c0, c3)) & (idx < np.maximum(c0, c3))
    body = np.where(mask, u * np.float32(imm2), -FMAX).astype(np.float32)
    return body, _rowmax(body, c1)


_mask_c3 = (Idx >= minn(C0, C3)) & (Idx < maxx(C0, C3))

_OP_DEFS = [
    ("TRIPLET_MAXLT2", Spec(
        body=select(Src0 + Src1 < (Zero - C0), Src0 + Src1, MaxNeg),
        accum=maxx, accum_init=C1,
        reference=_ref_maxlt2)),
    ("TRIPLET_WINMAX", Spec(
        body=_spill_c3_to_src1(select(_mask_c3, Src0 * C2, MaxNeg)),
        accum=maxx, accum_init=C1,
        reference=_ref_winmax)),
]

_REGISTERED = {}


def _register_ops():
    if _REGISTERED:
        return _REGISTERED
    ver = dve_ver_for("TRN2")
    for name, spec in _OP_DEFS:
        op = _dops.DveOp(name, spec, subdim=False, uops_sha={})
        _dops._SUB_OPCODE_FOR_NAME[name] = max(
            _dops._SUB_OPCODE_FOR_NAME.values()) + 1
        assert _dops._SUB_OPCODE_FOR_NAME[name] < 0x20
        # pin the sha: compile once to learn it, then accept it
        try:
            op.compile(ver)
        except ValueError as e:
            m = re.search(r"(\w+): lower\(\) output drifted \(\w+: (\w+)", str(e))
            assert m, f"unexpected sha error: {e}"
            op.uops_sha[ver] = m.group(2)
        op.compile(ver)
        _dops.OPS.append(op)
        _dops.CUSTOM_DVE_SPECS[name] = spec
        _REGISTERED[name] = op
    return _REGISTERED


# column layout of the per-row metadata tensor rowv[128, NF*NB]
F_C0W, F_C1W = range(2)
NF = 2

NX = 8  # x-chunks per core (every block mines column-chunk NM-1 via custom)


def _gx_for_core(k: int, w: int) -> int:
    """The column macro-chunk mined by the custom op: must avoid every
    block-diagonal (class columns) of this core's 8 row blocks."""
    lo_g = max(0, ROWS * k - (w - 1)) // MCH
    hi_g = min(B - 1, ROWS * k + (NB - 1) * PB + PB - 1 + (w - 1) + PB) // MCH
    diag = set(range(lo_g, hi_g + 1))
    for cand in ((k // 2 + 2) % NM, (k // 2 + 3) % NM, (k // 2 + 1) % NM):
        if cand not in diag:
            return cand
    raise AssertionError(f"no diag-free chunk for core {k} (W={w})")


def _build_program(use_f32r: bool, W: int = W):
    ops = _register_ops()
    op_maxlt2 = ops["TRIPLET_MAXLT2"]
    op_winmax = ops["TRIPLET_WINMAX"]

    nc = bacc.Bacc("TRN2", target_bir_lowering=False, debug=False)

    mmdt = mybir.dt.float32r if use_f32r else F32

    # Column layout (host-permuted per core): chunks 0..NM-2 are "z" columns,
    # chunk NM-1 holds the core's diag-free "x" columns.  The x chunk is
    # mined by the custom op for blocks 0..NX-1; block NB-1 covers it via
    # the z path instead, so every (row, column) pair is mined exactly once.
    d_embT = nc.dram_tensor("embT", [D, B], mmdt, kind="ExternalInput").ap()
    # statwin: per-block-interleaved [stat_b | win_b] so one DMA covers a
    # group of blocks' window inputs; nsqnp packs [nsqn | ones | nsqnw].
    SWB = PB + W
    d_statwin = nc.dram_tensor(
        "statwin", [D, NB * SWB], mmdt, kind="ExternalInput").ap()
    d_nsqnp = nc.dram_tensor(
        "nsqnp", [1, B + PB + NB * W], mmdt, kind="ExternalInput").ap()
    d_nsqbx = nc.dram_tensor("nsqbx", [PB, MCH], F32, kind="ExternalInput").ap()
    d_rowv = nc.dram_tensor("rowv", [PB, NF * NB], F32, kind="ExternalInput").ap()
    d_out = nc.dram_tensor("out", [PB, 3 * NB], F32, kind="ExternalOutput").ap()

    with tile.TileContext(nc) as tc, ExitStack() as ctx:
        big = ctx.enter_context(tc.tile_pool(name="big", bufs=1))
        med = ctx.enter_context(tc.tile_pool(name="med", bufs=1))
        sm = ctx.enter_context(tc.tile_pool(name="sm", bufs=2))
        wzp = ctx.enter_context(tc.tile_pool(name="wzp", bufs=4))
        chk = ctx.enter_context(tc.tile_pool(name="chk", bufs=2))
        # z-pipeline: 3 x 2-bank buffers; x-path: its own 2-bank buffer
        psz = ctx.enter_context(tc.tile_pool(name="psz", bufs=3, space="PSUM"))
        psx = ctx.enter_context(tc.tile_pool(name="psx", bufs=1, space="PSUM"))

        # ---- persistent SBUF inputs (window tensors first, then the z
        # column chunks of embT, then the x-path tensors) ----
        # Input streaming ordered to match consumption deadlines: the first
        # embT z-column chunk and blocks 0-1 window inputs lead (mining
        # starts ~5us in); later window groups arrive just ahead of their
        # woven windows; the x-column tensors and remaining chunks follow.
        embT = big.tile([D, B], mmdt, tag="embT")
        rowv = med.tile([PB, NF * NB], F32, tag="rowv")
        statwin = med.tile([D, NB * SWB], mmdt, tag="statwin")
        nsqnp = med.tile([1, B + PB + NB * W], mmdt, tag="nsqnp")
        nc.sync.dma_start(statwin[:, 0:2 * SWB], d_statwin[:, 0:2 * SWB])
        nc.sync.dma_start(nsqnp[:], d_nsqnp[:])
        nc.sync.dma_start(rowv[:], d_rowv[:])
        nsqn1 = nsqnp[0:1, 0:B]
        ones1 = nsqnp[0:1, B:B + PB]
        nsqnw1 = nsqnp[0:1, B + PB:]
        nc.sync.dma_start(embT[:, 0:MCH // 2], d_embT[:, 0:MCH // 2])
        nc.sync.dma_start(embT[:, MCH // 2:MCH], d_embT[:, MCH // 2:MCH])
        nc.sync.dma_start(statwin[:, 2 * SWB:4 * SWB],
                          d_statwin[:, 2 * SWB:4 * SWB])
        nc.sync.dma_start(statwin[:, 4 * SWB:], d_statwin[:, 4 * SWB:])
        nsqbx = med.tile([PB, MCH], F32, tag="nsqbx")
        for g in (NM - 1, 1, 2):
            nc.sync.dma_start(
                embT[:, g * MCH:(g + 1) * MCH], d_embT[:, g * MCH:(g + 1) * MCH])
            if g == NM - 1:
                nc.sync.dma_start(nsqbx[:], d_nsqbx[:])

        def stat_b(b):
            return statwin[:, b * SWB: b * SWB + PB]

        def win_b(b):
            return statwin[:, b * SWB + PB: (b + 1) * SWB]

        # dummy activation on an already-resident tile: forces the Ln
        # act-table load at ~1us, off the first real z-chunk's critical path
        dummy = sm.tile([PB, 1], F32, tag="dummy")
        nc.scalar.activation(dummy[:], rowv[:, 0:1], ACT.Ln,
                             bias=rowv[:, 0:1], scale=1.0)

        outv = med.tile([PB, 3 * NB], F32, tag="outv")
        nc.gpsimd.memset(outv[:], NEG_INIT)

        def rv(f, b):
            return rowv[:, f * NB + b: f * NB + b + 1]

        hps = outv[:, 0:NB]          # hp_t per block (written by winmax)
        nhps = med.tile([PB, NB], F32, tag="nhps")   # uL - EPS (Act Ln bias)

        # ---- fused schedule: windows are woven into the mining stream so
        # the Act engine can start as soon as block 0's window is mined.
        # z-work (Act Ln + DVE 2x min-accum) covers column chunks 0..NM-2
        # for every block; x-work (custom DVE) covers chunk NM-1.  Both run
        # at HCH (2-bank) granularity through separate PSUM pools so the
        # slower custom-DVE consumption never starves the Act pipeline. ----
        HCH = MCH // 2
        assert W <= HCH

        def emit_win(b):
            wp = psz.tile([PB, HCH], F32, tag="psz")
            nc.tensor.matmul(
                wp[:, 0:W], lhsT=stat_b(b), rhs=win_b(b),
                start=True, stop=False,
            )
            nc.tensor.matmul(
                wp[:, 0:W], lhsT=ones1,
                rhs=nsqnp[0:1, B + PB + b * W: B + PB + (b + 1) * W],
                start=False, stop=True,
            )
            wscr = sm.tile([PB, W], F32, tag="wscr")
            nc.vector._custom_dve(
                op_winmax, out=wscr[:], in0=wp[:, 0:W],
                in1=rv(F_C1W, b),
                s0=rv(F_C0W, b), s1=NEG_INIT, imm2=-1.0,
                accum_out=hps[:, b: b + 1],
            )
            # per-block Act bias: nhps_b = -hp_t - EPS = uL - EPS
            nc.vector.tensor_scalar(
                out=nhps[:, b: b + 1], in0=hps[:, b: b + 1],
                scalar1=-1.0, scalar2=-EPS, op0=ALU.mult, op1=ALU.add,
            )

        xseen = [False] * NB

        def emit_x(b, h):
            lhsT = stat_b(b)
            ps = psx.tile([PB, HCH], F32, tag="psx")
            for c in range(HCH // CH):
                col = (NM - 1) * MCH + h * HCH + c * CH
                nc.tensor.matmul(
                    ps[:, c * CH:(c + 1) * CH], lhsT=lhsT,
                    rhs=embT[:, col:col + CH],
                    start=True, stop=True,
                )
            scr = chk.tile([PB, HCH], F32, tag="scr")
            seed = outv[:, NB + b: NB + b + 1] if xseen[b] else NEG_INIT
            nc.vector._custom_dve(
                op_maxlt2, out=scr[:], in0=ps[:],
                in1=nsqbx[:, h * HCH:(h + 1) * HCH],
                s0=hps[:, b: b + 1], s1=seed,
                accum_out=outv[:, NB + b: NB + b + 1],
            )
            xseen[b] = True

        zseen = [False] * NB

        def emit_z(b, g, h):
            lhsT = stat_b(b)
            ps = psz.tile([PB, HCH], F32, tag="psz")
            for c in range(HCH // CH):
                col = g * MCH + h * HCH + c * CH
                nc.tensor.matmul(
                    ps[:, c * CH:(c + 1) * CH], lhsT=lhsT,
                    rhs=embT[:, col:col + CH],
                    start=True, stop=False,
                )
                nc.tensor.matmul(
                    ps[:, c * CH:(c + 1) * CH], lhsT=ones1,
                    rhs=nsqnp[0:1, col:col + CH],
                    start=False, stop=True,
                )
            wz = wzp.tile([PB, HCH], F32, tag="wz")
            nc.scalar.activation(wz[:], ps[:], ACT.Ln,
                                 bias=nhps[:, b: b + 1], scale=-1.0)
            zscr = chk.tile([PB, HCH], F32, tag="zscr")
            seed = outv[:, 2 * NB + b: 2 * NB + b + 1] if zseen[b] else FMAX
            nc.vector.tensor_scalar(
                out=zscr[:], in0=wz[:], scalar1=seed, scalar2=None,
                op0=ALU.min, op1=ALU.min,
                accum_out=outv[:, 2 * NB + b: 2 * NB + b + 1],
            )
            zseen[b] = True

        # x-halves, consumed from this queue and woven into the z stream
        xq = [(b, h) for b in range(NB) for h in range(MCH // HCH)]
        xi = 0

        # column 0: windows woven in PAIRS after the preceding block's z
        # work, so a late window-input DMA never blocks queued DVE work
        emit_win(0)
        emit_win(1)
        for b in range(NB):
            emit_z(b, 0, 0)
            emit_z(b, 0, 1)
            if b % 2 == 1:
                for j in (b + 1, b + 2):
                    if 2 <= j < NB:
                        emit_win(j)
            if b >= 6 and xi < len(xq):
                emit_x(*xq[xi])
                xi += 1
        # columns 1..NM-2 with one x-half per two z-halves (x leads each
        # pair so the stream drains with a cheap ts, not a custom op)
        for g in range(1, NM - 1):
            for b in range(NB):
                for h in range(MCH // HCH):
                    if (b * (MCH // HCH) + h) % 2 == 0 and xi < len(xq):
                        emit_x(*xq[xi])
                        xi += 1
                    emit_z(b, g, h)
        while xi < len(xq):
            emit_x(*xq[xi])
            xi += 1

        nc.sync.dma_start(d_out[:], outv[:])

    nc.compile()
    return nc


def _sort_and_stats(emb, labels):
    order = np.argsort(labels, kind="stable")
    embS = np.ascontiguousarray(emb[order])
    labS = np.asarray(labels[order])
    sqn = np.einsum("ij,ij->i", embS, embS, dtype=np.float32).astype(np.float32)
    uniq, first = np.unique(labS, return_index=True)
    ends = np.concatenate([first[1:], [B]]).astype(np.int64)
    cls_of_row = np.searchsorted(uniq, labS)
    c0 = first[cls_of_row].astype(np.int64)
    c1 = ends[cls_of_row].astype(np.int64)
    return embS, sqn, c0, c1


def _prep_inputs(embS, sqn, c0, c1, W: int = W):
    embT = np.ascontiguousarray(embS.T)           # [D, B]
    nsq = (-sqn).astype(np.float32)               # [B]
    nsqn = nsq[None, :]

    in_maps = []
    for k in range(NCORES):
        r0 = k * ROWS
        gx = _gx_for_core(k, W)
        # permute columns so the diag-free x chunk sits at position NM-1
        cord = [g for g in range(NM) if g != gx] + [gx]
        colperm = np.concatenate(
            [np.arange(g * MCH, (g + 1) * MCH) for g in cord])
        embT_k = np.ascontiguousarray(embT[:, colperm])
        nsqn_k = np.ascontiguousarray(nsqn[:, colperm])
        SWB = PB + W
        statwin = np.empty((D, NB * SWB), np.float32)
        nsqnp = np.empty((1, B + PB + NB * W), np.float32)
        nsqnp[0, 0:B] = nsqn_k[0]
        nsqnp[0, B:B + PB] = 1.0
        rowv = np.empty((PB, NF * NB), np.float32)
        for b in range(NB):
            g0 = r0 + b * PB
            lo = int(c0[g0])
            hi = int(c1[g0 + PB - 1])
            assert hi - lo <= W, f"window too small: {hi - lo} > {W}"
            w = min(lo, B - W)
            statwin[:, b * SWB: b * SWB + PB] = \
                2.0 * embT[:, g0: g0 + PB]
            statwin[:, b * SWB + PB: (b + 1) * SWB] = embT[:, w: w + W]
            nsqnp[0, B + PB + b * W: B + PB + (b + 1) * W] = nsqn[0, w: w + W]
            rows = np.arange(g0, g0 + PB)
            rowv[:, F_C0W * NB + b] = c0[rows] - w
            rowv[:, F_C1W * NB + b] = c1[rows] - w
        in_maps.append(
            {
                "embT": embT_k,
                "statwin": statwin,
                "nsqnp": nsqnp,
                "nsqbx": np.ascontiguousarray(np.broadcast_to(
                    nsqn_k[:, (NM - 1) * MCH:], (PB, MCH))),
                "rowv": rowv,
            }
        )
    return in_maps


def _finalize_host(embS, sqn, c0, c1, hp_t, m1, zw):
    """Per-row epilogue in numpy (f32), mirroring the reference semantics.

    hp_t: [B] window max of -u;  m1: [B] x-chunk max{u:u<uL} (or <=NEG_INIT);
    zw: [B] z-chunk min ln(uL-u) (FMAX-seeded; NaN impossible by seeding).
    """
    hp_sq = (hp_t + sqn).astype(np.float32)
    n_class = (c1 - c0)
    has_neg = n_class < B
    valid = (hp_sq > 0) & has_neg & (n_class > 1)
    hp = np.sqrt(np.maximum(hp_sq, 0, dtype=np.float32)).astype(np.float32)
    uL = (-hp_t).astype(np.float32)

    m = np.full(B, -FMAX, np.float32)
    rescue = np.zeros(B, bool)

    has1 = m1 > -1.0e37
    m = np.where(has1, m1, m)

    zw64 = zw.astype(np.float64)
    zok = np.isfinite(zw) & (zw < LN_NONE)
    rescue |= (zw <= LN_TINY) | np.isneginf(zw)
    gap = np.exp(np.where(zok & ~rescue, zw64, 0.0)) + np.float64(EPS)
    m2 = (uL.astype(np.float64) - gap).astype(np.float32)
    use2 = zok & ~rescue
    m = np.where(use2 & (m2 > m), m2, m)

    zz = (np.float32(2 * MARGIN) * hp + np.float32(MARGIN * MARGIN)).astype(
        np.float32)
    negUt = (uL - zz).astype(np.float32)
    semi_ex = m > negUt

    semi_u = np.where(semi_ex, m, np.float32(0.0)).astype(np.float32)
    fb = (valid & ~semi_ex) | (valid & rescue)

    per_row = np.zeros(B, np.float32)
    ok = valid & ~fb
    semi_sq = (sqn - semi_u).astype(np.float32)
    semi_d = np.sqrt(np.maximum(semi_sq, 0, dtype=np.float32)).astype(np.float32)
    per_row[ok] = np.maximum(hp[ok] - semi_d[ok] + np.float32(MARGIN), 0)

    big = np.float32(FMAX)
    for i in np.nonzero(fb)[0]:
        # exact per-row recompute, mirroring the reference
        u_row = (
            2.0 * (embS @ embS[i].astype(np.float32)).astype(np.float32) - sqn
        ).astype(np.float32)
        sq = np.maximum(sqn[i] - u_row, 0, dtype=np.float32)
        d = np.sqrt(sq).astype(np.float32)
        neg = np.ones(B, bool)
        neg[c0[i]:c1[i]] = False
        hpi = hp[i]
        semi_mask = neg & (d > hpi) & (d < hpi + np.float32(MARGIN))
        if semi_mask.any():
            sd = d[semi_mask].min()
        else:
            sd = d[neg].min() if neg.any() else big
        per_row[i] = max(hpi - sd + np.float32(MARGIN), 0.0)

    count = float(valid.sum())
    total = float(per_row[valid].sum(dtype=np.float64))
    return np.float32(total / max(count, 1.0) if count > 0 else 0.0)


def run(emb, labels, profile=False, use_f32r=True):
    emb = np.ascontiguousarray(np.asarray(emb, dtype=np.float32))
    labels = np.asarray(labels)
    assert emb.shape == (B, D), emb.shape
    embS, sqn, c0, c1 = _sort_and_stats(emb, labels)

    # window must cover the widest per-block class span
    worst = max(
        int(c1[g0 + PB - 1] - c0[g0]) for g0 in range(0, B, PB)
    )
    w = W
    while w < worst:
        w *= 2
    assert w <= 2048, f"class span {worst} too wide"

    key = (bool(use_f32r), w)
    if key not in _PROGRAM_CACHE:
        _PROGRAM_CACHE[key] = _build_program(use_f32r, w)
    nc = _PROGRAM_CACHE[key]

    in_maps = _prep_inputs(embS, sqn, c0, c1, w)
    res = run_bass_kernel_spmd(
        nc, in_maps, list(range(NCORES)), trace=profile
    )
    hp_t = np.empty(B, np.float32)
    m1 = np.full(B, -FMAX, np.float32)
    zw = np.full(B, FMAX, np.float32)
    for k, r in enumerate(res.results):
        o = r["out"]                      # [PB, 3*NB]
        for b in range(NB):
            g0 = k * ROWS + b * PB
            hp_t[g0: g0 + PB] = o[:, b]
            if b < NX:
                m1[g0: g0 + PB] = o[:, NB + b]
            zw[g0: g0 + PB] = o[:, 2 * NB + b]
    loss = _finalize_host(embS, sqn, c0, c1, hp_t, m1, zw)
    return loss, res


def kernel(emb, labels):
    loss, _ = run(emb, labels, profile=False,
                  use_f32r=os.environ.get("TRIPLET_F32R", "1") == "1")
    return np.array(loss, dtype=np.float32)


# revision 41
# speedup vs baseline: 1.0418x; 1.0418x over previous
"""Batch semi-hard triplet loss on 8 Trainium2 NeuronCores (Bass/Tile).

Strategy (anchor-row sharding, per sharding hint):
  - Host sorts rows by label (loss is permutation invariant), computes row
    norms and per-row class-block bounds [c0, c1) in sorted order.
  - Each core mines its [1024, 8192] stripe of u = 2*x_i.x_j - |x_j|^2
    (sq dist = |x_i|^2 - u).  Work is split across THREE engines per
    128-row block, 2048-col macro chunk:
      * window phase (hardest positive): PE window matmul + rank-1 norm,
        one custom-DVE TRIPLET_WINMAX per block over the class window
        [c0, c1) (self column included: it only lowers hp_sq to exactly 0
        for singleton classes, which the host invalidates anyway).
        hp_t[b] = max over window of -u, so uL = -hp_t = min u over class.
      * x-chunks (one fixed 2048-col chunk per core, chosen away from the
        block-diagonal): custom-DVE TRIPLET_MAXLT2 reads the dot-only PSUM
        and adds the column norms via its elementwise Src1 stream
        (select(Src0+Src1 < -C0) -> max).  No rank-1 matmul needed.
      * z-chunks (the rest): PE rank-1 completes u in PSUM, the Act engine
        computes w = Ln(uL - u) (excluded cols u >= uL give NaN, which the
        DVE reduce suppresses - hardware-verified semantics), and the DVE
        folds w with a 2x-mode tensor_scalar(min,min) accum chained across
        chunks.  Host decodes gap = exp(min w), maxLT = uL - gap.
  - Device returns per-row (hp_t, x-chunk maxLT, z-chunk min-ln-gap).
    Host finishes the tiny per-row math; rows whose decoded gap is
    suspiciously tiny (exact-hit/leak degenerates) and rows with an empty
    semi-hard band are recomputed exactly on the host (~tens of rows).
"""

import os
import re
import sys

for _p in (
    "/root/.axon_site/_ro/trn_rl_repo/concourse",
    "/root/.axon_site/_ro/trn_rl_repo",
    "/root/.axon_site/_ro/pypackages",
):
    if _p not in sys.path:
        sys.path.insert(0, _p)

from contextlib import ExitStack

import numpy as np

import mybir
import concourse.bass as bass
import concourse.bacc as bacc
import concourse.tile as tile
from concourse.bass_utils import run_bass_kernel_spmd
from concourse import dve_ops as _dops
from concourse.dve_spec import (
    C0, C1, C2, C3, Idx, MaxNeg, Spec, Src0, Src1, Zero, maxx, minn, select,
    _spill_c3_to_src1,
)
from concourse.dve_table_gen import dve_ver_for

B = 8192
D = 128
NCORES = 8
ROWS = B // NCORES        # rows per core
PB = 128                  # rows per block (partition dim)
NB = ROWS // PB           # blocks per core
CH = 512                  # one PSUM bank of f32 (matmul moving max)
MCH = 2048                # macro chunk (4 banks)
NM = B // MCH             # macro chunks across the full column range
W = 256                   # positive-mining window (auto-doubles if needed)
MARGIN = 0.3
NEG_INIT = -3.0e38
FMAX = float(np.finfo(np.float32).max)
LN_NONE = 20.0            # min-ln above this => no semi-hard candidate
LN_TINY = -25.0           # min-ln below this => host rescue (coincidence)
EPS = 1e-3                # guard band: Act arg = (uL - EPS) - u, so the
                          # bit-exact min-positive (u == uL) lands at -EPS
                          # -> NaN -> suppressed; host adds EPS back.

F32 = mybir.dt.float32
F16 = mybir.dt.float16
AX = mybir.AxisListType
ALU = mybir.AluOpType
ACT = mybir.ActivationFunctionType

_PROGRAM_CACHE = {}

# ---------------------------------------------------------------------------
# custom DVE ops
# ---------------------------------------------------------------------------


def _rowmax(body, init):
    m = body.reshape(body.shape[0], -1).max(axis=-1, keepdims=True)
    return np.maximum(np.asarray(init, np.float32).reshape(-1, 1) * np.ones_like(m), m)


def _ref_maxlt2(in0, in1, c0, c1, imm2):
    u = (in0.astype(np.float32) + in1.astype(np.float32)).astype(np.float32)
    thr = (-np.asarray(c0, np.float32)).reshape(-1, 1)
    body = np.where(u < thr, u, -FMAX).astype(np.float32)
    return body, _rowmax(body, c1)


def _ref_winmax(in0, in1, c0, c1, imm2):
    # in1 carries the spilled C3 (mask end), [P, 1]
    u = in0.astype(np.float32)
    c3 = in1.reshape(in1.shape[0], 1)
    idx = np.arange(u.shape[-1], dtype=np.float32)
    mask = (idx >= np.minimum(c0, c3)) & (idx < np.maximum(c0, c3))
    body = np.where(mask, u * np.float32(imm2), -FMAX).astype(np.float32)
    return body, _rowmax(body, c1)


_mask_c3 = (Idx >= minn(C0, C3)) & (Idx < maxx(C0, C3))

_OP_DEFS = [
    ("TRIPLET_MAXLT2", Spec(
        body=select(Src0 + Src1 < (Zero - C0), Src0 + Src1, MaxNeg),
        accum=maxx, accum_init=C1,
        reference=_ref_maxlt2)),
    ("TRIPLET_WINMAX", Spec(
        body=_spill_c3_to_src1(select(_mask_c3, Src0 * C2, MaxNeg)),
        accum=maxx, accum_init=C1,
        reference=_ref_winmax)),
]

_REGISTERED = {}


def _register_ops():
    if _REGISTERED:
        return _REGISTERED
    ver = dve_ver_for("TRN2")
    for name, spec in _OP_DEFS:
        op = _dops.DveOp(name, spec, subdim=False, uops_sha={})
        _dops._SUB_OPCODE_FOR_NAME[name] = max(
            _dops._SUB_OPCODE_FOR_NAME.values()) + 1
        assert _dops._SUB_OPCODE_FOR_NAME[name] < 0x20
        # pin the sha: compile once to learn it, then accept it
        try:
            op.compile(ver)
        except ValueError as e:
            m = re.search(r"(\w+): lower\(\) output drifted \(\w+: (\w+)", str(e))
            assert m, f"unexpected sha error: {e}"
            op.uops_sha[ver] = m.group(2)
        op.compile(ver)
        _dops.OPS.append(op)
        _dops.CUSTOM_DVE_SPECS[name] = spec
        _REGISTERED[name] = op
    return _REGISTERED


# column layout of the per-row metadata tensor rowv[128, NF*NB]
F_C0W, F_C1W = range(2)
NF = 2

NX = 8  # x-chunks per core (every block mines column-chunk NM-1 via custom)


def _cord_for_core(k: int, w: int) -> list:
    """Column-chunk permutation order for core k: the chunks containing any
    of this core's block-diagonals (class columns) go to positions 0..1
    (z-path only); the diag-free chunks fill positions 2..3, which the
    custom-DVE x-path mines."""
    lo_g = max(0, ROWS * k - (w - 1)) // MCH
    hi_g = min(B - 1, ROWS * k + ROWS - 1 + (w - 1) + PB) // MCH
    diag = list(range(lo_g, hi_g + 1))
    free = [g for g in range(NM) if g not in diag]
    assert len(free) >= 2, f"core {k}: too few diag-free chunks (W={w})"
    cord = diag + free[:-2] + free[-2:]
    assert len(cord) == NM
    return cord


def _build_program(use_f32r: bool, W: int = W):
    ops = _register_ops()
    op_maxlt2 = ops["TRIPLET_MAXLT2"]
    op_winmax = ops["TRIPLET_WINMAX"]

    nc = bacc.Bacc("TRN2", target_bir_lowering=False, debug=False)

    mmdt = mybir.dt.float32r if use_f32r else F32

    # Column layout (host-permuted per core): chunks 0..NM-2 are "z" columns,
    # chunk NM-1 holds the core's diag-free "x" columns.  The x chunk is
    # mined by the custom op for blocks 0..NX-1; block NB-1 covers it via
    # the z path instead, so every (row, column) pair is mined exactly once.
    d_embT = nc.dram_tensor("embT", [D, B], mmdt, kind="ExternalInput").ap()
    # statwin: per-block-interleaved [stat_b | win_b] so one DMA covers a
    # group of blocks' window inputs; nsqnp packs [nsqn | ones | nsqnw].
    SWB = PB + W
    d_statwin = nc.dram_tensor(
        "statwin", [D, NB * SWB], mmdt, kind="ExternalInput").ap()
    d_nsqnp = nc.dram_tensor(
        "nsqnp", [1, B + PB + NB * W], mmdt, kind="ExternalInput").ap()
    d_nsqbx = nc.dram_tensor("nsqbx", [PB, 2 * MCH], F32, kind="ExternalInput").ap()
    d_rowv = nc.dram_tensor("rowv", [PB, NF * NB], F32, kind="ExternalInput").ap()
    d_out = nc.dram_tensor("out", [PB, 3 * NB], F32, kind="ExternalOutput").ap()

    with tile.TileContext(nc) as tc, ExitStack() as ctx:
        big = ctx.enter_context(tc.tile_pool(name="big", bufs=1))
        med = ctx.enter_context(tc.tile_pool(name="med", bufs=1))
        sm = ctx.enter_context(tc.tile_pool(name="sm", bufs=2))
        wzp = ctx.enter_context(tc.tile_pool(name="wzp", bufs=4))
        chk = ctx.enter_context(tc.tile_pool(name="chk", bufs=2))
        # z-pipeline: 2 x 2-bank buffers; x-path: 2 x 2-bank buffers
        psz = ctx.enter_context(tc.tile_pool(name="psz", bufs=2, space="PSUM"))
        psx = ctx.enter_context(tc.tile_pool(name="psx", bufs=2, space="PSUM"))

        # ---- persistent SBUF inputs (window tensors first, then the z
        # column chunks of embT, then the x-path tensors) ----
        # Input streaming ordered to match consumption deadlines: the first
        # embT z-column chunk and blocks 0-1 window inputs lead (mining
        # starts ~5us in); later window groups arrive just ahead of their
        # woven windows; the x-column tensors and remaining chunks follow.
        embT = big.tile([D, B], mmdt, tag="embT")
        rowv = med.tile([PB, NF * NB], F32, tag="rowv")
        statwin = med.tile([D, NB * SWB], mmdt, tag="statwin")
        nsqnp = med.tile([1, B + PB + NB * W], mmdt, tag="nsqnp")
        nc.sync.dma_start(statwin[:, 0:2 * SWB], d_statwin[:, 0:2 * SWB])
        nc.sync.dma_start(nsqnp[:], d_nsqnp[:])
        nc.sync.dma_start(rowv[:], d_rowv[:])
        nsqn1 = nsqnp[0:1, 0:B]
        ones1 = nsqnp[0:1, B:B + PB]
        nsqnw1 = nsqnp[0:1, B + PB:]
        nc.sync.dma_start(embT[:, 0:MCH // 2], d_embT[:, 0:MCH // 2])
        nc.sync.dma_start(embT[:, MCH // 2:MCH], d_embT[:, MCH // 2:MCH])
        nc.sync.dma_start(statwin[:, 2 * SWB:4 * SWB],
                          d_statwin[:, 2 * SWB:4 * SWB])
        nc.sync.dma_start(statwin[:, 4 * SWB:], d_statwin[:, 4 * SWB:])
        nsqbx = med.tile([PB, 2 * MCH], F32, tag="nsqbx")
        nc.sync.dma_start(embT[:, 3 * MCH:4 * MCH], d_embT[:, 3 * MCH:4 * MCH])
        nc.sync.dma_start(nsqbx[:, MCH:], d_nsqbx[:, MCH:])
        nc.sync.dma_start(embT[:, 1 * MCH:2 * MCH], d_embT[:, 1 * MCH:2 * MCH])
        nc.sync.dma_start(embT[:, 2 * MCH:3 * MCH], d_embT[:, 2 * MCH:3 * MCH])
        nc.sync.dma_start(nsqbx[:, 0:MCH], d_nsqbx[:, 0:MCH])

        def stat_b(b):
            return statwin[:, b * SWB: b * SWB + PB]

        def win_b(b):
            return statwin[:, b * SWB + PB: (b + 1) * SWB]

        # dummy activation on an already-resident tile: forces the Ln
        # act-table load at ~1us, off the first real z-chunk's critical path
        dummy = sm.tile([PB, 1], F32, tag="dummy")
        nc.scalar.activation(dummy[:], rowv[:, 0:1], ACT.Ln,
                             bias=rowv[:, 0:1], scale=1.0)

        outv = med.tile([PB, 3 * NB], F32, tag="outv")
        nc.gpsimd.memset(outv[:], NEG_INIT)

        def rv(f, b):
            return rowv[:, f * NB + b: f * NB + b + 1]

        hps = outv[:, 0:NB]          # hp_t per block (written by winmax)
        nhps = med.tile([PB, NB], F32, tag="nhps")   # uL - EPS (Act Ln bias)

        # ---- fused schedule: windows are woven into the mining stream so
        # the Act engine can start as soon as block 0's window is mined.
        # z-work (Act Ln + DVE 2x min-accum) covers column chunks 0..NM-2
        # for every block; x-work (custom DVE) covers chunk NM-1.  Both run
        # at HCH (2-bank) granularity through separate PSUM pools so the
        # slower custom-DVE consumption never starves the Act pipeline. ----
        HCH = MCH // 2
        assert W <= HCH

        def emit_win(b):
            wp = psz.tile([PB, HCH], F32, tag="psz")
            nc.tensor.matmul(
                wp[:, 0:W], lhsT=stat_b(b), rhs=win_b(b),
                start=True, stop=False,
            )
            nc.tensor.matmul(
                wp[:, 0:W], lhsT=ones1,
                rhs=nsqnp[0:1, B + PB + b * W: B + PB + (b + 1) * W],
                start=False, stop=True,
            )
            wscr = sm.tile([PB, W], F32, tag="wscr")
            nc.vector._custom_dve(
                op_winmax, out=wscr[:], in0=wp[:, 0:W],
                in1=rv(F_C1W, b),
                s0=rv(F_C0W, b), s1=NEG_INIT, imm2=-1.0,
                accum_out=hps[:, b: b + 1],
            )
            # per-block Act bias: nhps_b = -hp_t - EPS = uL - EPS
            nc.vector.tensor_scalar(
                out=nhps[:, b: b + 1], in0=hps[:, b: b + 1],
                scalar1=-1.0, scalar2=-EPS, op0=ALU.mult, op1=ALU.add,
            )

        xseen = [False] * NB

        def emit_x(b, pos, h):
            lhsT = stat_b(b)
            ps = psx.tile([PB, HCH], F32, tag="psx")
            for c in range(HCH // CH):
                col = pos * MCH + h * HCH + c * CH
                nc.tensor.matmul(
                    ps[:, c * CH:(c + 1) * CH], lhsT=lhsT,
                    rhs=embT[:, col:col + CH],
                    start=True, stop=True,
                )
            scr = chk.tile([PB, HCH], F32, tag="scr")
            seed = outv[:, NB + b: NB + b + 1] if xseen[b] else NEG_INIT
            nc.vector._custom_dve(
                op_maxlt2, out=scr[:], in0=ps[:],
                in1=nsqbx[:, (pos - 2) * MCH + h * HCH:
                           (pos - 2) * MCH + (h + 1) * HCH],
                s0=hps[:, b: b + 1], s1=seed,
                accum_out=outv[:, NB + b: NB + b + 1],
            )
            xseen[b] = True

        zseen = [False] * NB

        def emit_z(b, g, h):
            lhsT = stat_b(b)
            ps = psz.tile([PB, HCH], F32, tag="psz")
            for c in range(HCH // CH):
                col = g * MCH + h * HCH + c * CH
                nc.tensor.matmul(
                    ps[:, c * CH:(c + 1) * CH], lhsT=lhsT,
                    rhs=embT[:, col:col + CH],
                    start=True, stop=False,
                )
                nc.tensor.matmul(
                    ps[:, c * CH:(c + 1) * CH], lhsT=ones1,
                    rhs=nsqnp[0:1, col:col + CH],
                    start=False, stop=True,
                )
            wz = wzp.tile([PB, HCH], F16, tag="wz")
            nc.scalar.activation(wz[:], ps[:], ACT.Ln,
                                 bias=nhps[:, b: b + 1], scale=-1.0)
            zscr = chk.tile([PB, HCH], F16, tag="zscr")
            seed = outv[:, 2 * NB + b: 2 * NB + b + 1] if zseen[b] else FMAX
            nc.vector.tensor_scalar(
                out=zscr[:], in0=wz[:], scalar1=seed, scalar2=None,
                op0=ALU.min, op1=ALU.min,
                accum_out=outv[:, 2 * NB + b: 2 * NB + b + 1],
            )
            zseen[b] = True

        # z stream: positions 0..1 for all blocks plus position 2 for
        # blocks 3..NB-1; x stream (custom DVE): position 3 for all blocks
        # plus position 2 for blocks 0..2.  x-halves are woven
        # proportionally once their columns + nsqbx have streamed in.
        zq = [(b, 0, h) for b in range(NB) for h in range(MCH // HCH)]
        zq += [(b, 1, h) for b in range(NB) for h in range(MCH // HCH)]
        zq += [(b, 2, h) for b in range(3, NB) for h in range(MCH // HCH)]
        xq = [(b, 3, h) for b in range(NB) for h in range(MCH // HCH)]
        xq += [(b, 2, h) for b in range(3) for h in range(MCH // HCH)]
        xi = 0
        Z0 = 7                       # first x after this many z-halves
        RATE = len(xq) / (len(zq) - Z0)

        emit_win(0)
        emit_win(1)
        for zi, (b, g, h) in enumerate(zq):
            if zi >= Z0:
                while xi < len(xq) and xi < (zi - Z0 + 1) * RATE:
                    emit_x(*xq[xi])
                    xi += 1
            emit_z(b, g, h)
            if g == 0 and h == 1 and b % 2 == 1:
                for j in (b + 1, b + 2):
                    if 2 <= j < NB:
                        emit_win(j)
        while xi < len(xq):
            emit_x(*xq[xi])
            xi += 1

        nc.sync.dma_start(d_out[:], outv[:])

    nc.compile()
    return nc


def _sort_and_stats(emb, labels):
    order = np.argsort(labels, kind="stable")
    embS = np.ascontiguousarray(emb[order])
    labS = np.asarray(labels[order])
    sqn = np.einsum("ij,ij->i", embS, embS, dtype=np.float32).astype(np.float32)
    uniq, first = np.unique(labS, return_index=True)
    ends = np.concatenate([first[1:], [B]]).astype(np.int64)
    cls_of_row = np.searchsorted(uniq, labS)
    c0 = first[cls_of_row].astype(np.int64)
    c1 = ends[cls_of_row].astype(np.int64)
    return embS, sqn, c0, c1


def _prep_inputs(embS, sqn, c0, c1, W: int = W):
    embT = np.ascontiguousarray(embS.T)           # [D, B]
    nsq = (-sqn).astype(np.float32)               # [B]
    nsqn = nsq[None, :]

    in_maps = []
    for k in range(NCORES):
        r0 = k * ROWS
        # permute columns so diag chunks sit at positions 0..1 and the
        # custom-mined (x-path) chunks at positions 2..3
        cord = _cord_for_core(k, W)
        colperm = np.concatenate(
            [np.arange(g * MCH, (g + 1) * MCH) for g in cord])
        embT_k = np.ascontiguousarray(embT[:, colperm])
        nsqn_k = np.ascontiguousarray(nsqn[:, colperm])
        SWB = PB + W
        statwin = np.empty((D, NB * SWB), np.float32)
        nsqnp = np.empty((1, B + PB + NB * W), np.float32)
        nsqnp[0, 0:B] = nsqn_k[0]
        nsqnp[0, B:B + PB] = 1.0
        rowv = np.empty((PB, NF * NB), np.float32)
        for b in range(NB):
            g0 = r0 + b * PB
            lo = int(c0[g0])
            hi = int(c1[g0 + PB - 1])
            assert hi - lo <= W, f"window too small: {hi - lo} > {W}"
            w = min(lo, B - W)
            statwin[:, b * SWB: b * SWB + PB] = \
                2.0 * embT[:, g0: g0 + PB]
            statwin[:, b * SWB + PB: (b + 1) * SWB] = embT[:, w: w + W]
            nsqnp[0, B + PB + b * W: B + PB + (b + 1) * W] = nsqn[0, w: w + W]
            rows = np.arange(g0, g0 + PB)
            rowv[:, F_C0W * NB + b] = c0[rows] - w
            rowv[:, F_C1W * NB + b] = c1[rows] - w
        in_maps.append(
            {
                "embT": embT_k,
                "statwin": statwin,
                "nsqnp": nsqnp,
                "nsqbx": np.ascontiguousarray(np.broadcast_to(
                    nsqn_k[:, (NM - 2) * MCH:], (PB, 2 * MCH))),
                "rowv": rowv,
            }
        )
    return in_maps


def _finalize_host(embS, sqn, c0, c1, hp_t, m1, zw):
    """Per-row epilogue in numpy (f32), mirroring the reference semantics.

    hp_t: [B] window max of -u;  m1: [B] x-chunk max{u:u<uL} (or <=NEG_INIT);
    zw: [B] z-chunk min ln(uL-u) (FMAX-seeded; NaN impossible by seeding).
    """
    hp_sq = (hp_t + sqn).astype(np.float32)
    n_class = (c1 - c0)
    has_neg = n_class < B
    valid = (hp_sq > 0) & has_neg & (n_class > 1)
    hp = np.sqrt(np.maximum(hp_sq, 0, dtype=np.float32)).astype(np.float32)
    uL = (-hp_t).astype(np.float32)

    m = np.full(B, -FMAX, np.float32)
    rescue = np.zeros(B, bool)

    has1 = m1 > -1.0e37
    m = np.where(has1, m1, m)

    zw64 = zw.astype(np.float64)
    zok = np.isfinite(zw) & (zw < LN_NONE)
    rescue |= (zw <= LN_TINY) | np.isneginf(zw)
    gap = np.exp(np.where(zok & ~rescue, zw64, 0.0)) + np.float64(EPS)
    m2 = (uL.astype(np.float64) - gap).astype(np.float32)
    use2 = zok & ~rescue
    m = np.where(use2 & (m2 > m), m2, m)

    zz = (np.float32(2 * MARGIN) * hp + np.float32(MARGIN * MARGIN)).astype(
        np.float32)
    negUt = (uL - zz).astype(np.float32)
    semi_ex = m > negUt

    semi_u = np.where(semi_ex, m, np.float32(0.0)).astype(np.float32)
    fb = (valid & ~semi_ex) | (valid & rescue)

    per_row = np.zeros(B, np.float32)
    ok = valid & ~fb
    semi_sq = (sqn - semi_u).astype(np.float32)
    semi_d = np.sqrt(np.maximum(semi_sq, 0, dtype=np.float32)).astype(np.float32)
    per_row[ok] = np.maximum(hp[ok] - semi_d[ok] + np.float32(MARGIN), 0)

    big = np.float32(FMAX)
    for i in np.nonzero(fb)[0]:
        # exact per-row recompute, mirroring the reference
        u_row = (
            2.0 * (embS @ embS[i].astype(np.float32)).astype(np.float32) - sqn
        ).astype(np.float32)
        sq = np.maximum(sqn[i] - u_row, 0, dtype=np.float32)
        d = np.sqrt(sq).astype(np.float32)
        neg = np.ones(B, bool)
        neg[c0[i]:c1[i]] = False
        hpi = hp[i]
        semi_mask = neg & (d > hpi) & (d < hpi + np.float32(MARGIN))
        if semi_mask.any():
            sd = d[semi_mask].min()
        else:
            sd = d[neg].min() if neg.any() else big
        per_row[i] = max(hpi - sd + np.float32(MARGIN), 0.0)

    count = float(valid.sum())
    total = float(per_row[valid].sum(dtype=np.float64))
    return np.float32(total / max(count, 1.0) if count > 0 else 0.0)


def run(emb, labels, profile=False, use_f32r=True):
    emb = np.ascontiguousarray(np.asarray(emb, dtype=np.float32))
    labels = np.asarray(labels)
    assert emb.shape == (B, D), emb.shape
    embS, sqn, c0, c1 = _sort_and_stats(emb, labels)

    # window must cover the widest per-block class span
    worst = max(
        int(c1[g0 + PB - 1] - c0[g0]) for g0 in range(0, B, PB)
    )
    w = W
    while w < worst:
        w *= 2
    assert w <= 2048, f"class span {worst} too wide"

    key = (bool(use_f32r), w)
    if key not in _PROGRAM_CACHE:
        _PROGRAM_CACHE[key] = _build_program(use_f32r, w)
    nc = _PROGRAM_CACHE[key]

    in_maps = _prep_inputs(embS, sqn, c0, c1, w)
    res = run_bass_kernel_spmd(
        nc, in_maps, list(range(NCORES)), trace=profile
    )
    hp_t = np.empty(B, np.float32)
    m1 = np.full(B, -FMAX, np.float32)
    zw = np.full(B, FMAX, np.float32)
    for k, r in enumerate(res.results):
        o = r["out"]                      # [PB, 3*NB]
        for b in range(NB):
            g0 = k * ROWS + b * PB
            hp_t[g0: g0 + PB] = o[:, b]
            if b < NX:
                m1[g0: g0 + PB] = o[:, NB + b]
            zw[g0: g0 + PB] = o[:, 2 * NB + b]
    loss = _finalize_host(embS, sqn, c0, c1, hp_t, m1, zw)
    return loss, res


def kernel(emb, labels):
    loss, _ = run(emb, labels, profile=False,
                  use_f32r=os.environ.get("TRIPLET_F32R", "1") == "1")
    return np.array(loss, dtype=np.float32)


# revision 48
# speedup vs baseline: 1.0613x; 1.0187x over previous
"""Batch semi-hard triplet loss on 8 Trainium2 NeuronCores (Bass/Tile).

Strategy (anchor-row sharding, per sharding hint):
  - Host sorts rows by label (loss is permutation invariant), computes row
    norms and per-row class-block bounds [c0, c1) in sorted order.
  - Each core mines its [1024, 8192] stripe of u = 2*x_i.x_j - |x_j|^2
    (sq dist = |x_i|^2 - u).  Work is split across THREE engines per
    128-row block, 2048-col macro chunk:
      * window phase (hardest positive): PE window matmul + rank-1 norm,
        one custom-DVE TRIPLET_WINMAX per block over the class window
        [c0, c1) (self column included: it only lowers hp_sq to exactly 0
        for singleton classes, which the host invalidates anyway).
        hp_t[b] = max over window of -u, so uL = -hp_t = min u over class.
      * x-chunks (one fixed 2048-col chunk per core, chosen away from the
        block-diagonal): custom-DVE TRIPLET_MAXLT2 reads the dot-only PSUM
        and adds the column norms via its elementwise Src1 stream
        (select(Src0+Src1 < -C0) -> max).  No rank-1 matmul needed.
      * z-chunks (the rest): PE rank-1 completes u in PSUM, the Act engine
        computes w = Ln(uL - u) (excluded cols u >= uL give NaN, which the
        DVE reduce suppresses - hardware-verified semantics), and the DVE
        folds w with a 2x-mode tensor_scalar(min,min) accum chained across
        chunks.  Host decodes gap = exp(min w), maxLT = uL - gap.
  - Device returns per-row (hp_t, x-chunk maxLT, z-chunk min-ln-gap).
    Host finishes the tiny per-row math; rows whose decoded gap is
    suspiciously tiny (exact-hit/leak degenerates) and rows with an empty
    semi-hard band are recomputed exactly on the host (~tens of rows).
"""

import os
import re
import sys

for _p in (
    "/root/.axon_site/_ro/trn_rl_repo/concourse",
    "/root/.axon_site/_ro/trn_rl_repo",
    "/root/.axon_site/_ro/pypackages",
):
    if _p not in sys.path:
        sys.path.insert(0, _p)

from contextlib import ExitStack

import numpy as np

import mybir
import concourse.bass as bass
import concourse.bacc as bacc
import concourse.tile as tile
from concourse.bass_utils import run_bass_kernel_spmd
from concourse import dve_ops as _dops
from concourse.dve_spec import (
    C0, C1, C2, C3, Idx, MaxNeg, Spec, Src0, Src1, Zero, maxx, minn, select,
    _spill_c3_to_src1,
)
from concourse.dve_table_gen import dve_ver_for

B = 8192
D = 128
NCORES = 8
ROWS = B // NCORES        # rows per core
PB = 128                  # rows per block (partition dim)
NB = ROWS // PB           # blocks per core
CH = 512                  # one PSUM bank of f32 (matmul moving max)
MCH = 2048                # macro chunk (4 banks)
NM = B // MCH             # macro chunks across the full column range
W = 256                   # positive-mining window (auto-doubles if needed)
MARGIN = 0.3
NEG_INIT = -3.0e38
FMAX = float(np.finfo(np.float32).max)
LN_NONE = 20.0            # min-ln above this => no semi-hard candidate
LN_TINY = -25.0           # min-ln below this => host rescue (coincidence)
EPS = 1e-3                # guard band: Act arg = (uL - EPS) - u, so the
                          # bit-exact min-positive (u == uL) lands at -EPS
                          # -> NaN -> suppressed; host adds EPS back.

F32 = mybir.dt.float32
F16 = mybir.dt.float16
AX = mybir.AxisListType
ALU = mybir.AluOpType
ACT = mybir.ActivationFunctionType

_PROGRAM_CACHE = {}

# ---------------------------------------------------------------------------
# custom DVE ops
# ---------------------------------------------------------------------------


def _rowmax(body, init):
    m = body.reshape(body.shape[0], -1).max(axis=-1, keepdims=True)
    return np.maximum(np.asarray(init, np.float32).reshape(-1, 1) * np.ones_like(m), m)


def _ref_maxlt2(in0, in1, c0, c1, imm2):
    u = (in0.astype(np.float32) + in1.astype(np.float32)).astype(np.float32)
    thr = (-np.asarray(c0, np.float32)).reshape(-1, 1)
    body = np.where(u < thr, u, -FMAX).astype(np.float32)
    return body, _rowmax(body, c1)


def _ref_winmax(in0, in1, c0, c1, imm2):
    # in1 carries the spilled C3 (mask end), [P, 1]
    u = in0.astype(np.float32)
    c3 = in1.reshape(in1.shape[0], 1)
    idx = np.arange(u.shape[-1], dtype=np.float32)
    mask = (idx >= np.minimum(c0, c3)) & (idx < np.maximum(c0, c3))
    body = np.where(mask, u * np.float32(imm2), -FMAX).astype(np.float32)
    return body, _rowmax(body, c1)


_mask_c3 = (Idx >= minn(C0, C3)) & (Idx < maxx(C0, C3))

_OP_DEFS = [
    ("TRIPLET_MAXLT2", Spec(
        body=select(Src0 + Src1 < (Zero - C0), Src0 + Src1, MaxNeg),
        accum=maxx, accum_init=C1,
        reference=_ref_maxlt2)),
    ("TRIPLET_WINMAX", Spec(
        body=_spill_c3_to_src1(select(_mask_c3, Src0 * C2, MaxNeg)),
        accum=maxx, accum_init=C1,
        reference=_ref_winmax)),
]

_REGISTERED = {}


def _register_ops():
    if _REGISTERED:
        return _REGISTERED
    ver = dve_ver_for("TRN2")
    for name, spec in _OP_DEFS:
        op = _dops.DveOp(name, spec, subdim=False, uops_sha={})
        _dops._SUB_OPCODE_FOR_NAME[name] = max(
            _dops._SUB_OPCODE_FOR_NAME.values()) + 1
        assert _dops._SUB_OPCODE_FOR_NAME[name] < 0x20
        # pin the sha: compile once to learn it, then accept it
        try:
            op.compile(ver)
        except ValueError as e:
            m = re.search(r"(\w+): lower\(\) output drifted \(\w+: (\w+)", str(e))
            assert m, f"unexpected sha error: {e}"
            op.uops_sha[ver] = m.group(2)
        op.compile(ver)
        _dops.OPS.append(op)
        _dops.CUSTOM_DVE_SPECS[name] = spec
        _REGISTERED[name] = op
    return _REGISTERED


# column layout of the per-row metadata tensor rowv[128, NF*NB]
F_C0W, F_C1W = range(2)
NF = 2

NX = 8  # x-chunks per core (every block mines column-chunk NM-1 via custom)


def _cord_for_core(k: int, w: int) -> list:
    """Column-chunk permutation order for core k: the chunks containing any
    of this core's block-diagonals (class columns) go to positions 0..1
    (z-path only); the diag-free chunks fill positions 2..3, which the
    custom-DVE x-path mines."""
    lo_g = max(0, ROWS * k - (w - 1)) // MCH
    hi_g = min(B - 1, ROWS * k + ROWS - 1 + (w - 1) + PB) // MCH
    diag = list(range(lo_g, hi_g + 1))
    free = [g for g in range(NM) if g not in diag]
    assert len(free) >= 2, f"core {k}: too few diag-free chunks (W={w})"
    cord = diag + free[:-2] + free[-2:]
    assert len(cord) == NM
    return cord


def _build_program(use_f32r: bool, W: int = W):
    ops = _register_ops()
    op_maxlt2 = ops["TRIPLET_MAXLT2"]
    op_winmax = ops["TRIPLET_WINMAX"]

    nc = bacc.Bacc("TRN2", target_bir_lowering=False, debug=False)

    mmdt = mybir.dt.float32r if use_f32r else F32

    # Column layout (host-permuted per core): chunks 0..NM-2 are "z" columns,
    # chunk NM-1 holds the core's diag-free "x" columns.  The x chunk is
    # mined by the custom op for blocks 0..NX-1; block NB-1 covers it via
    # the z path instead, so every (row, column) pair is mined exactly once.
    d_embT = nc.dram_tensor("embT", [D, B], mmdt, kind="ExternalInput").ap()
    # statwin: per-block-interleaved [stat_b | win_b] so one DMA covers a
    # group of blocks' window inputs; nsqnp packs [nsqn | ones | nsqnw].
    SWB = PB + W
    d_statwin = nc.dram_tensor(
        "statwin", [D, NB * SWB], mmdt, kind="ExternalInput").ap()
    d_nsqnp = nc.dram_tensor(
        "nsqnp", [1, B + PB + NB * W], mmdt, kind="ExternalInput").ap()
    d_nsqbx = nc.dram_tensor("nsqbx", [PB, 2 * MCH], F32, kind="ExternalInput").ap()
    d_rowv = nc.dram_tensor("rowv", [PB, NF * NB], F32, kind="ExternalInput").ap()
    d_out = nc.dram_tensor("out", [PB, 3 * NB], F32, kind="ExternalOutput").ap()

    with tile.TileContext(nc) as tc, ExitStack() as ctx:
        big = ctx.enter_context(tc.tile_pool(name="big", bufs=1))
        med = ctx.enter_context(tc.tile_pool(name="med", bufs=1))
        sm = ctx.enter_context(tc.tile_pool(name="sm", bufs=2))
        wzp = ctx.enter_context(tc.tile_pool(name="wzp", bufs=4))
        chk = ctx.enter_context(tc.tile_pool(name="chk", bufs=2))
        # z-pipeline: 2 x 2-bank buffers; x-path: 2 x 2-bank buffers
        psz = ctx.enter_context(tc.tile_pool(name="psz", bufs=2, space="PSUM"))
        psx = ctx.enter_context(tc.tile_pool(name="psx", bufs=2, space="PSUM"))

        # ---- persistent SBUF inputs (window tensors first, then the z
        # column chunks of embT, then the x-path tensors) ----
        # Input streaming ordered to match consumption deadlines: the first
        # embT z-column chunk and blocks 0-1 window inputs lead (mining
        # starts ~5us in); later window groups arrive just ahead of their
        # woven windows; the x-column tensors and remaining chunks follow.
        embT = big.tile([D, B], mmdt, tag="embT")
        rowv = med.tile([PB, NF * NB], F32, tag="rowv")
        statwin = med.tile([D, NB * SWB], mmdt, tag="statwin")
        nsqnp = med.tile([1, B + PB + NB * W], mmdt, tag="nsqnp")
        nc.sync.dma_start(statwin[:, 0:2 * SWB], d_statwin[:, 0:2 * SWB])
        nc.sync.dma_start(nsqnp[:], d_nsqnp[:])
        nc.sync.dma_start(rowv[:], d_rowv[:])
        nsqn1 = nsqnp[0:1, 0:B]
        ones1 = nsqnp[0:1, B:B + PB]
        nsqnw1 = nsqnp[0:1, B + PB:]
        nc.sync.dma_start(embT[:, 0:MCH // 2], d_embT[:, 0:MCH // 2])
        nc.sync.dma_start(embT[:, MCH // 2:MCH], d_embT[:, MCH // 2:MCH])
        nc.sync.dma_start(statwin[:, 2 * SWB:4 * SWB],
                          d_statwin[:, 2 * SWB:4 * SWB])
        nc.sync.dma_start(statwin[:, 4 * SWB:], d_statwin[:, 4 * SWB:])
        nsqbx = med.tile([PB, 2 * MCH], F32, tag="nsqbx")
        nc.sync.dma_start(embT[:, 3 * MCH:4 * MCH], d_embT[:, 3 * MCH:4 * MCH])
        nc.sync.dma_start(nsqbx[:, MCH:], d_nsqbx[:, MCH:])
        nc.sync.dma_start(embT[:, 1 * MCH:2 * MCH], d_embT[:, 1 * MCH:2 * MCH])
        nc.sync.dma_start(embT[:, 2 * MCH:3 * MCH], d_embT[:, 2 * MCH:3 * MCH])
        nc.sync.dma_start(nsqbx[:, 0:MCH], d_nsqbx[:, 0:MCH])

        def stat_b(b):
            return statwin[:, b * SWB: b * SWB + PB]

        def win_b(b):
            return statwin[:, b * SWB + PB: (b + 1) * SWB]

        # dummy activation on an already-resident tile: forces the Ln
        # act-table load at ~1us, off the first real z-chunk's critical path
        dummy = sm.tile([PB, 1], F32, tag="dummy")
        nc.scalar.activation(dummy[:], rowv[:, 0:1], ACT.Ln,
                             bias=rowv[:, 0:1], scale=1.0)

        outv = med.tile([PB, 3 * NB], F32, tag="outv")
        nc.gpsimd.memset(outv[:], NEG_INIT)

        def rv(f, b):
            return rowv[:, f * NB + b: f * NB + b + 1]

        hps = outv[:, 0:NB]          # hp_t per block (written by winmax)
        nhps = med.tile([PB, NB], F32, tag="nhps")   # uL - EPS (Act Ln bias)

        # ---- fused schedule: windows are woven into the mining stream so
        # the Act engine can start as soon as block 0's window is mined.
        # z-work (Act Ln + DVE 2x min-accum) covers column chunks 0..NM-2
        # for every block; x-work (custom DVE) covers chunk NM-1.  Both run
        # at HCH (2-bank) granularity through separate PSUM pools so the
        # slower custom-DVE consumption never starves the Act pipeline. ----
        HCH = MCH // 2
        assert W <= HCH

        def emit_win(b):
            wp = psz.tile([PB, HCH], F32, tag="psz")
            nc.tensor.matmul(
                wp[:, 0:W], lhsT=stat_b(b), rhs=win_b(b),
                start=True, stop=False,
            )
            nc.tensor.matmul(
                wp[:, 0:W], lhsT=ones1,
                rhs=nsqnp[0:1, B + PB + b * W: B + PB + (b + 1) * W],
                start=False, stop=True,
            )
            wscr = sm.tile([PB, W], F32, tag="wscr")
            nc.vector._custom_dve(
                op_winmax, out=wscr[:], in0=wp[:, 0:W],
                in1=rv(F_C1W, b),
                s0=rv(F_C0W, b), s1=NEG_INIT, imm2=-1.0,
                accum_out=hps[:, b: b + 1],
            )
            # per-block Act bias: nhps_b = -hp_t - EPS = uL - EPS
            nc.vector.tensor_scalar(
                out=nhps[:, b: b + 1], in0=hps[:, b: b + 1],
                scalar1=-1.0, scalar2=-EPS, op0=ALU.mult, op1=ALU.add,
            )

        xseen = [False] * NB

        def emit_x(b, pos, h):
            lhsT = stat_b(b)
            ps = psx.tile([PB, HCH], F32, tag="psx")
            for c in range(HCH // CH):
                col = pos * MCH + h * HCH + c * CH
                nc.tensor.matmul(
                    ps[:, c * CH:(c + 1) * CH], lhsT=lhsT,
                    rhs=embT[:, col:col + CH],
                    start=True, stop=True,
                )
            scr = chk.tile([PB, HCH], F32, tag="scr")
            seed = outv[:, NB + b: NB + b + 1] if xseen[b] else NEG_INIT
            nc.vector._custom_dve(
                op_maxlt2, out=scr[:], in0=ps[:],
                in1=nsqbx[:, (pos - 2) * MCH + h * HCH:
                           (pos - 2) * MCH + (h + 1) * HCH],
                s0=hps[:, b: b + 1], s1=seed,
                accum_out=outv[:, NB + b: NB + b + 1],
            )
            xseen[b] = True

        zseen = [False] * NB

        def emit_z(b, g, h):
            lhsT = stat_b(b)
            ps = psz.tile([PB, HCH], F32, tag="psz")
            for c in range(HCH // CH):
                col = g * MCH + h * HCH + c * CH
                nc.tensor.matmul(
                    ps[:, c * CH:(c + 1) * CH], lhsT=lhsT,
                    rhs=embT[:, col:col + CH],
                    start=True, stop=False,
                )
                nc.tensor.matmul(
                    ps[:, c * CH:(c + 1) * CH], lhsT=ones1,
                    rhs=nsqnp[0:1, col:col + CH],
                    start=False, stop=True,
                )
            wz = wzp.tile([PB, HCH], F16, tag="wz")
            nc.scalar.activation(wz[:], ps[:], ACT.Ln,
                                 bias=nhps[:, b: b + 1], scale=-1.0)
            zscr = chk.tile([PB, HCH], F16, tag="zscr")
            seed = outv[:, 2 * NB + b: 2 * NB + b + 1] if zseen[b] else FMAX
            nc.vector.tensor_scalar(
                out=zscr[:], in0=wz[:], scalar1=seed, scalar2=None,
                op0=ALU.min, op1=ALU.min,
                accum_out=outv[:, 2 * NB + b: 2 * NB + b + 1],
            )
            zseen[b] = True

        # z stream: positions 0..1 for all blocks plus position 2 for
        # blocks 3..NB-1; x stream (custom DVE): position 3 for all blocks
        # plus position 2 for blocks 0..2.  x-halves are woven
        # proportionally once their columns + nsqbx have streamed in.
        XB2 = 4   # blocks 0..XB2-1 mine position 2 via the custom op too
        zq = [(b, 0, h) for b in range(NB) for h in range(MCH // HCH)]
        zq += [(b, 1, h) for b in range(NB) for h in range(MCH // HCH)]
        zq += [(b, 2, h) for b in range(XB2, NB) for h in range(MCH // HCH)]
        xq = [(b, 3, h) for b in range(NB) for h in range(MCH // HCH)]
        xq += [(b, 2, h) for b in range(XB2) for h in range(MCH // HCH)]
        xi = 0
        Z0 = 7                       # first x after this many z-halves
        RATE = len(xq) / (len(zq) - Z0)

        emit_win(0)
        emit_win(1)
        for zi, (b, g, h) in enumerate(zq):
            if zi >= Z0:
                while xi < len(xq) and xi < (zi - Z0 + 1) * RATE:
                    emit_x(*xq[xi])
                    xi += 1
            emit_z(b, g, h)
            if g == 0 and h == 1 and b % 2 == 1:
                for j in (b + 1, b + 2):
                    if 2 <= j < NB:
                        emit_win(j)
        while xi < len(xq):
            emit_x(*xq[xi])
            xi += 1

        nc.sync.dma_start(d_out[:], outv[:])

    nc.compile()
    return nc


def _sort_and_stats(emb, labels):
    order = np.argsort(labels, kind="stable")
    embS = np.ascontiguousarray(emb[order])
    labS = np.asarray(labels[order])
    sqn = np.einsum("ij,ij->i", embS, embS, dtype=np.float32).astype(np.float32)
    uniq, first = np.unique(labS, return_index=True)
    ends = np.concatenate([first[1:], [B]]).astype(np.int64)
    cls_of_row = np.searchsorted(uniq, labS)
    c0 = first[cls_of_row].astype(np.int64)
    c1 = ends[cls_of_row].astype(np.int64)
    return embS, sqn, c0, c1


def _prep_inputs(embS, sqn, c0, c1, W: int = W):
    embT = np.ascontiguousarray(embS.T)           # [D, B]
    nsq = (-sqn).astype(np.float32)               # [B]
    nsqn = nsq[None, :]

    in_maps = []
    for k in range(NCORES):
        r0 = k * ROWS
        # permute columns so diag chunks sit at positions 0..1 and the
        # custom-mined (x-path) chunks at positions 2..3
        cord = _cord_for_core(k, W)
        colperm = np.concatenate(
            [np.arange(g * MCH, (g + 1) * MCH) for g in cord])
        embT_k = np.ascontiguousarray(embT[:, colperm])
        nsqn_k = np.ascontiguousarray(nsqn[:, colperm])
        SWB = PB + W
        statwin = np.empty((D, NB * SWB), np.float32)
        nsqnp = np.empty((1, B + PB + NB * W), np.float32)
        nsqnp[0, 0:B] = nsqn_k[0]
        nsqnp[0, B:B + PB] = 1.0
        rowv = np.empty((PB, NF * NB), np.float32)
        for b in range(NB):
            g0 = r0 + b * PB
            lo = int(c0[g0])
            hi = int(c1[g0 + PB - 1])
            assert hi - lo <= W, f"window too small: {hi - lo} > {W}"
            w = min(lo, B - W)
            statwin[:, b * SWB: b * SWB + PB] = \
                2.0 * embT[:, g0: g0 + PB]
            statwin[:, b * SWB + PB: (b + 1) * SWB] = embT[:, w: w + W]
            nsqnp[0, B + PB + b * W: B + PB + (b + 1) * W] = nsqn[0, w: w + W]
            rows = np.arange(g0, g0 + PB)
            rowv[:, F_C0W * NB + b] = c0[rows] - w
            rowv[:, F_C1W * NB + b] = c1[rows] - w
        in_maps.append(
            {
                "embT": embT_k,
                "statwin": statwin,
                "nsqnp": nsqnp,
                "nsqbx": np.ascontiguousarray(np.broadcast_to(
                    nsqn_k[:, (NM - 2) * MCH:], (PB, 2 * MCH))),
                "rowv": rowv,
            }
        )
    return in_maps


def _finalize_host(embS, sqn, c0, c1, hp_t, m1, zw):
    """Per-row epilogue in numpy (f32), mirroring the reference semantics.

    hp_t: [B] window max of -u;  m1: [B] x-chunk max{u:u<uL} (or <=NEG_INIT);
    zw: [B] z-chunk min ln(uL-u) (FMAX-seeded; NaN impossible by seeding).
    """
    hp_sq = (hp_t + sqn).astype(np.float32)
    n_class = (c1 - c0)
    has_neg = n_class < B
    valid = (hp_sq > 0) & has_neg & (n_class > 1)
    hp = np.sqrt(np.maximum(hp_sq, 0, dtype=np.float32)).astype(np.float32)
    uL = (-hp_t).astype(np.float32)

    m = np.full(B, -FMAX, np.float32)
    rescue = np.zeros(B, bool)

    has1 = m1 > -1.0e37
    m = np.where(has1, m1, m)

    zw64 = zw.astype(np.float64)
    zok = np.isfinite(zw) & (zw < LN_NONE)
    rescue |= (zw <= LN_TINY) | np.isneginf(zw)
    gap = np.exp(np.where(zok & ~rescue, zw64, 0.0)) + np.float64(EPS)
    m2 = (uL.astype(np.float64) - gap).astype(np.float32)
    use2 = zok & ~rescue
    m = np.where(use2 & (m2 > m), m2, m)

    zz = (np.float32(2 * MARGIN) * hp + np.float32(MARGIN * MARGIN)).astype(
        np.float32)
    negUt = (uL - zz).astype(np.float32)
    semi_ex = m > negUt

    semi_u = np.where(semi_ex, m, np.float32(0.0)).astype(np.float32)
    fb = (valid & ~semi_ex) | (valid & rescue)

    per_row = np.zeros(B, np.float32)
    ok = valid & ~fb
    semi_sq = (sqn - semi_u).astype(np.float32)
    semi_d = np.sqrt(np.maximum(semi_sq, 0, dtype=np.float32)).astype(np.float32)
    per_row[ok] = np.maximum(hp[ok] - semi_d[ok] + np.float32(MARGIN), 0)

    big = np.float32(FMAX)
    for i in np.nonzero(fb)[0]:
        # exact per-row recompute, mirroring the reference
        u_row = (
            2.0 * (embS @ embS[i].astype(np.float32)).astype(np.float32) - sqn
        ).astype(np.float32)
        sq = np.maximum(sqn[i] - u_row, 0, dtype=np.float32)
        d = np.sqrt(sq).astype(np.float32)
        neg = np.ones(B, bool)
        neg[c0[i]:c1[i]] = False
        hpi = hp[i]
        semi_mask = neg & (d > hpi) & (d < hpi + np.float32(MARGIN))
        if semi_mask.any():
            sd = d[semi_mask].min()
        else:
            sd = d[neg].min() if neg.any() else big
        per_row[i] = max(hpi - sd + np.float32(MARGIN), 0.0)

    count = float(valid.sum())
    total = float(per_row[valid].sum(dtype=np.float64))
    return np.float32(total / max(count, 1.0) if count > 0 else 0.0)


def run(emb, labels, profile=False, use_f32r=True):
    emb = np.ascontiguousarray(np.asarray(emb, dtype=np.float32))
    labels = np.asarray(labels)
    assert emb.shape == (B, D), emb.shape
    embS, sqn, c0, c1 = _sort_and_stats(emb, labels)

    # window must cover the widest per-block class span
    worst = max(
        int(c1[g0 + PB - 1] - c0[g0]) for g0 in range(0, B, PB)
    )
    w = W
    while w < worst:
        w *= 2
    assert w <= 2048, f"class span {worst} too wide"

    key = (bool(use_f32r), w)
    if key not in _PROGRAM_CACHE:
        _PROGRAM_CACHE[key] = _build_program(use_f32r, w)
    nc = _PROGRAM_CACHE[key]

    in_maps = _prep_inputs(embS, sqn, c0, c1, w)
    res = run_bass_kernel_spmd(
        nc, in_maps, list(range(NCORES)), trace=profile
    )
    hp_t = np.empty(B, np.float32)
    m1 = np.full(B, -FMAX, np.float32)
    zw = np.full(B, FMAX, np.float32)
    for k, r in enumerate(res.results):
        o = r["out"]                      # [PB, 3*NB]
        for b in range(NB):
            g0 = k * ROWS + b * PB
            hp_t[g0: g0 + PB] = o[:, b]
            if b < NX:
                m1[g0: g0 + PB] = o[:, NB + b]
            zw[g0: g0 + PB] = o[:, 2 * NB + b]
    loss = _finalize_host(embS, sqn, c0, c1, hp_t, m1, zw)
    return loss, res


def kernel(emb, labels):
    loss, _ = run(emb, labels, profile=False,
                  use_f32r=os.environ.get("TRIPLET_F32R", "1") == "1")
    return np.array(loss, dtype=np.float32)
